# revision 20
# baseline (speedup 1.0000x reference)
"""Gated MLP (SwiGLU) on 8 TRN2 NeuronCores, tensor-parallel over the
intermediate dimension.

Math (per reference): g = x @ Wg.T ; u = x @ Wu.T ; a = silu(g)*u ;
d = a @ Wd.T, with x:[2,2048,4096] f32, Wg/Wu:[14336,4096], Wd:[4096,14336].

Sharding: core c owns intermediate slice I_c = c*1792:(c+1)*1792. Each core
computes gT/uT/aT for its slice against all 4096 tokens, then a partial
dT[c] = WdT[I_c,:].T-contraction. Host sums the 8 partials (the tp_reduce)
and transposes back.

On-chip layout (everything transposed so contractions land on partitions):
  xT  [H=4096, T=4096] bf16            (rhs for gate/up)
  wg/wu [14, 128, 4096] bf16 pre-tiled (lhsT [k128, i128] stationary;
                                        wg[i, p, k*128+m] = Wg.T[k*128+p, i*128+m])
  wd  [32, 128, 1792] bf16 pre-tiled   (lhsT [i128, h128] stationary)
  out [H, T] bf16 partial              (dT; host reduces in f32 + transposes)

Weight DMAs are contiguous per partition (pre-tiled on host) so each is a
single-block-per-partition SWDGE descriptor set. DMA issue is spread over
engines: xT on Sync, weights on Scalar, wd on GpSimd (first three on Vector,
mid-phase, to keep startup bandwidth for the critical xt/wg stream), outputs
on GpSimd (Sync for the last q, where that queue is idle and HWDGE latency
shortens the final drain).

Startup: the PE warms its HAM clock-gate with a short run of dummy matmuls
on a zeroed tile while the first real weight/activation chunks stream in,
so the real matmul stream starts at full clock.
"""

import sys

if "/opt/trn_rl_repo" not in sys.path:
    sys.path.insert(0, "/opt/trn_rl_repo")

import numpy as np
import ml_dtypes

H = 4096          # hidden
I_FULL = 14336    # intermediate
T = 4096          # tokens (2*2048)
NCORES = 8
ISH = I_FULL // NCORES   # 1792 per-core intermediate slice
P = 128
QT = 1024         # tokens per outer block
NQ = T // QT      # 4
KT = H // P       # 32 contraction tiles for gate/up
IT = ISH // P     # 14 contraction tiles for down
HT = H // P       # 32 output-row tiles for down
NF = 512          # matmul moving free-dim (one PSUM bank of f32)
NWARM = 36        # dummy matmuls to span preamble->first-data and warm HAM

_BUILT = {}


def _build():
    if "nc" in _BUILT:
        return _BUILT["nc"]
    from concourse import bacc
    import concourse.mybir as mybir
    import concourse.tile as tile
    from contextlib import ExitStack

    bf = mybir.dt.bfloat16
    f32 = mybir.dt.float32
    nc = bacc.Bacc(
        "TRN2",
        target_bir_lowering=False,
        debug=False,
        enable_asserts=False,
        num_devices=NCORES,
    )

    xT = nc.dram_tensor("xT", [H, T], bf, kind="ExternalInput").ap()
    wg = nc.dram_tensor("wg", [IT, P, KT * P], bf, kind="ExternalInput").ap()
    wu = nc.dram_tensor("wu", [IT, P, KT * P], bf, kind="ExternalInput").ap()
    wd = nc.dram_tensor("wd", [HT, P, IT * P], bf, kind="ExternalInput").ap()
    out = nc.dram_tensor("out", [H, T], bf, kind="ExternalOutput").ap()

    # [p, k, t] view: per-partition rows stay contiguous in t
    x_r = xT.rearrange("(k p) t -> p k t", p=P)     # [128, 32, 4096]

    with tile.TileContext(nc) as tc, ExitStack() as ctx:
        xt_pool = ctx.enter_context(tc.tile_pool(name="xt", bufs=KT + 2))
        wg_pool = ctx.enter_context(tc.tile_pool(name="wg", bufs=3))
        wu_pool = ctx.enter_context(tc.tile_pool(name="wu", bufs=2))
        wd_pool = ctx.enter_context(tc.tile_pool(name="wd", bufs=3))
        at_pool = ctx.enter_context(tc.tile_pool(name="at", bufs=IT))
        tmp_pool = ctx.enter_context(tc.tile_pool(name="tmp", bufs=2))
        dst_pool = ctx.enter_context(tc.tile_pool(name="dst", bufs=2))
        warm_pool = ctx.enter_context(tc.tile_pool(name="warm", bufs=1))
        pg_pool = ctx.enter_context(tc.tile_pool(name="pg", bufs=1, space="PSUM"))
        pu_pool = ctx.enter_context(tc.tile_pool(name="pu", bufs=1, space="PSUM"))
        pd_pool = ctx.enter_context(tc.tile_pool(name="pd", bufs=2, space="PSUM"))

        # ---- PE warmup: dummy matmuls on a zeroed tile while the first
        # real chunks stream from HBM. Keeps the PE busy from right after
        # the preamble barrier so the HAM clock-gate opens (K=8/8) before
        # the real stream begins, and hides the first-DMA latency. The
        # dummies target the first real pg tile (its first real matmul
        # restarts the accumulation group, so the garbage is discarded).
        warm_t = warm_pool.tile([P, P], bf)
        nc.vector.memset(warm_t[:], 0.0)
        pg0 = pg_pool.tile([P, QT], f32, name="pg")
        for w in range(NWARM):
            nc.tensor.matmul(
                pg0[:, 0:P], warm_t[:], warm_t[:], start=True, stop=True
            )

        def load_w(pool, src, i):
            t = pool.tile([P, KT, P], bf)
            # src[i] is [128, 4096] contiguous per partition
            nc.scalar.dma_start(out=t[:], in_=src[i].rearrange("p (k m) -> p k m", m=P))
            return t

        early_wd = {}

        for q in range(NQ):
            t0 = q * QT

            # first gate/up weights go out before the xT block so the PE can
            # start as soon as xt[0] lands; at kernel start, stage the first
            # TWO i-tiles' weights in interleaved k-chunks so every k-group
            # of the startup double-interleave has its weights early
            if q == 0:
                wg_t = wg_pool.tile([P, KT, P], bf)
                wu_t = wu_pool.tile([P, KT, P], bf)
                wg_t1 = wg_pool.tile([P, KT, P], bf, name="wg_t")
                wu_t1 = wu_pool.tile([P, KT, P], bf, name="wu_t")
                views = [
                    (wg_t, wg[0].rearrange("p (k m) -> p k m", m=P)),
                    (wu_t, wu[0].rearrange("p (k m) -> p k m", m=P)),
                    (wg_t1, wg[1].rearrange("p (k m) -> p k m", m=P)),
                    (wu_t1, wu[1].rearrange("p (k m) -> p k m", m=P)),
                ]
                for lo, hi in ((0, 2), (2, 6), (6, 14), (14, 23), (23, KT)):
                    for t, v in views:
                        nc.scalar.dma_start(out=t[:, lo:hi, :], in_=v[:, lo:hi, :])
            else:
                wg_t = load_w(wg_pool, wg, 0)
                wu_t = load_w(wu_pool, wu, 0)

            # stage this block's activations: 32 k-tiles of [128, QT]
            xts = []
            for k in range(KT):
                xt_t = xt_pool.tile([P, QT], bf)
                nc.sync.dma_start(out=xt_t[:], in_=x_r[:, k, t0 : t0 + QT])
                xts.append(xt_t)

            if q == 0:
                # first wd prefetches ride the sync ring BEHIND this q's xt
                # tiles: same-queue FIFO keeps them off the HBM-bound startup
                # window without a separate pacing mechanism
                for hh in range(3):
                    wd_t = wd_pool.tile([P, IT, P], bf, name="wd_t")
                    nc.sync.dma_start(
                        out=wd_t[:], in_=wd[hh].rearrange("p (i m) -> p i m", m=P)
                    )
                    early_wd[hh] = wd_t

            # ---- gate/up + silu*mul, producing aT[i] tiles ----
            ats = []
            if q == 0:
                # kernel start: the whole first pass is HBM-arrival-bound
                # (xt 8MB + weights must stream in). Interleave the first
                # TWO i-tiles per k so PE consumption (~1.7us/k-tile) stays
                # well behind arrival (~1.2us/k-tile) — no starvation, and
                # the HAM clock-gate stays open. i1's accumulators borrow
                # the down-projection PSUM slots, which are idle until the
                # down phase and match the tile shape exactly.
                pg = pg0
                pu = pu_pool.tile([P, QT], f32)
                pg1 = pd_pool.tile([P, QT], f32, name="pd")
                pu1 = pd_pool.tile([P, QT], f32, name="pd")
                tmp = tmp_pool.tile([P, QT], bf)
                tmp1 = tmp_pool.tile([P, QT], bf, name="tmp")
                for k in range(KT):
                    for w_t, ps in (
                        (wg_t, pg), (wu_t, pu), (wg_t1, pg1), (wu_t1, pu1)
                    ):
                        if k == KT - 1 and ps is pu:
                            # silu issued before the final u matmuls so the
                            # pg bank frees as early as possible for i=2
                            nc.scalar.activation(
                                tmp[:], pg[:], mybir.ActivationFunctionType.Silu
                            )
                        if k == KT - 1 and ps is pu1:
                            nc.scalar.activation(
                                tmp1[:], pg1[:], mybir.ActivationFunctionType.Silu
                            )
                        for n in range(QT // NF):
                            nc.tensor.matmul(
                                ps[:, n * NF : (n + 1) * NF],
                                w_t[:, k, :],
                                xts[k][:, n * NF : (n + 1) * NF],
                                start=(k == 0),
                                stop=(k == KT - 1),
                            )
                at = at_pool.tile([P, QT], bf)
                nc.vector.tensor_tensor(at[:], tmp[:], pu[:], mybir.AluOpType.mult)
                ats.append(at)
                at = at_pool.tile([P, QT], bf, name="at")
                nc.vector.tensor_tensor(at[:], tmp1[:], pu1[:], mybir.AluOpType.mult)
                ats.append(at)
            for i in range(0 if q else 2, IT):
                if q > 0 and i == 0:
                    pass
                else:
                    wg_t = load_w(wg_pool, wg, i)
                    wu_t = load_w(wu_pool, wu, i)
                pg = pg_pool.tile([P, QT], f32)
                if True:
                    for k in range(KT):
                        for n in range(QT // NF):
                            nc.tensor.matmul(
                                pg[:, n * NF : (n + 1) * NF],
                                wg_t[:, k, :],
                                xts[k][:, n * NF : (n + 1) * NF],
                                start=(k == 0),
                                stop=(k == KT - 1),
                            )
                    # silu(g) on ScalarE while the u matmuls run
                    tmp = tmp_pool.tile([P, QT], bf)
                    nc.scalar.activation(
                        tmp[:], pg[:], mybir.ActivationFunctionType.Silu
                    )
                    pu = pu_pool.tile([P, QT], f32)
                    for k in range(KT):
                        for n in range(QT // NF):
                            nc.tensor.matmul(
                                pu[:, n * NF : (n + 1) * NF],
                                wu_t[:, k, :],
                                xts[k][:, n * NF : (n + 1) * NF],
                                start=(k == 0),
                                stop=(k == KT - 1),
                            )
                at = at_pool.tile([P, QT], bf)
                nc.vector.tensor_tensor(
                    at[:], tmp[:], pu[:], mybir.AluOpType.mult
                )
                ats.append(at)

            # ---- down projection: dT[h, t] partial ----
            last_q = q == NQ - 1
            for h in range(HT):
                h0 = h * P
                if q == 0 and h in early_wd:
                    wd_t = early_wd.pop(h)
                else:
                    wd_t = wd_pool.tile([P, IT, P], bf)
                    nc.gpsimd.dma_start(
                        out=wd_t[:], in_=wd[h].rearrange("p (i m) -> p i m", m=P)
                    )
                pd = pd_pool.tile([P, QT], f32)
                if last_q and h == HT - 1:
                    # tail: finish the early chunks first so their copies +
                    # stores overlap the later matmuls; the final chunk is
                    # small (short HBM write-receipt) and sits in the other
                    # pd buffer so its matmuls don't share a PSUM bank with
                    # the chunk-2 copy
                    pd2 = pd_pool.tile([P, QT], f32, name="pd")
                    for c0, c1, ps in ((0, 512, pd), (512, 768, pd), (768, 1024, pd2)):
                        for i in range(IT):
                            nc.tensor.matmul(
                                ps[:, c0:c1],
                                wd_t[:, i, :],
                                ats[i][:, c0:c1],
                                start=(i == 0),
                                stop=(i == IT - 1),
                            )
                        dst = dst_pool.tile([P, NF], bf)
                        nc.vector.tensor_copy(dst[:, 0 : c1 - c0], ps[:, c0:c1])
                        nc.sync.dma_start(
                            out=out[h0 : h0 + P, t0 + c0 : t0 + c1],
                            in_=dst[:, 0 : c1 - c0],
                        )
                else:
                    for i in range(IT):
                        for n in range(QT // NF):
                            nc.tensor.matmul(
                                pd[:, n * NF : (n + 1) * NF],
                                wd_t[:, i, :],
                                ats[i][:, n * NF : (n + 1) * NF],
                                start=(i == 0),
                                stop=(i == IT - 1),
                            )
                    dst = dst_pool.tile([P, QT], bf)
                    nc.vector.tensor_copy(dst[:], pd[:])
                    # output DMAs on gpsimd so their waits don't stall input
                    # loads; on the last q the sync queue is idle and its
                    # HWDGE path has lower latency
                    eng = nc.sync if last_q else nc.gpsimd
                    eng.dma_start(
                        out=out[h0 : h0 + P, t0 : t0 + QT], in_=dst[:]
                    )

    nc.compile()
    _BUILT["nc"] = nc
    return nc


def _prep_inputs(x, Wg, Wu, Wd):
    bf = ml_dtypes.bfloat16
    xTn = x.reshape(T, H).T.astype(bf, order="C")        # [H, T]
    # single-pass cast + shard + pre-tile:
    #   wg[c][i, p, k*128+m] = Wg.T[k*128+p, c*1792 + i*128+m]
    wg_all = np.ascontiguousarray(
        Wg.reshape(NCORES, IT, P, KT, P).transpose(0, 1, 4, 3, 2), dtype=bf
    ).reshape(NCORES, IT, P, KT * P)
    wu_all = np.ascontiguousarray(
        Wu.reshape(NCORES, IT, P, KT, P).transpose(0, 1, 4, 3, 2), dtype=bf
    ).reshape(NCORES, IT, P, KT * P)
    #   wd[c][h, p, i*128+m] = Wd.T[c*1792 + i*128+p, h*128+m]
    wd_all = np.ascontiguousarray(
        Wd.reshape(HT, P, NCORES, IT, P).transpose(2, 0, 4, 3, 1), dtype=bf
    ).reshape(NCORES, HT, P, IT * P)
    return [
        {"xT": xTn, "wg": wg_all[c], "wu": wu_all[c], "wd": wd_all[c]}
        for c in range(NCORES)
    ]


def _run(in_maps, **kw):
    from concourse.bass_utils import run_bass_kernel_spmd

    nc = _build()
    return run_bass_kernel_spmd(nc, in_maps, core_ids=list(range(NCORES)), **kw)


def _gather(results, batch_shape):
    acc = results[0]["out"].astype(np.float32)
    for r in results[1:]:
        acc += r["out"].astype(np.float32)
    return np.ascontiguousarray(acc.T).reshape(batch_shape)


def kernel(x, Wg, Wu, Wd):
    x = np.asarray(x)
    in_maps = _prep_inputs(
        np.asarray(x, dtype=np.float32),
        np.asarray(Wg, dtype=np.float32),
        np.asarray(Wu, dtype=np.float32),
        np.asarray(Wd, dtype=np.float32),
    )
    res = _run(in_maps)
    return _gather(res.results, x.shape)


# revision 22
# speedup vs baseline: 1.0008x; 1.0008x over previous
"""Gated MLP (SwiGLU) on 8 TRN2 NeuronCores, tensor-parallel over the
intermediate dimension.

Math (per reference): g = x @ Wg.T ; u = x @ Wu.T ; a = silu(g)*u ;
d = a @ Wd.T, with x:[2,2048,4096] f32, Wg/Wu:[14336,4096], Wd:[4096,14336].

Sharding: core c owns intermediate slice I_c = c*1792:(c+1)*1792. Each core
computes gT/uT/aT for its slice against all 4096 tokens, then a partial
dT[c] = WdT[I_c,:].T-contraction. Host sums the 8 partials (the tp_reduce)
and transposes back.

On-chip layout (everything transposed so contractions land on partitions):
  xT  [H=4096, T=4096] bf16            (rhs for gate/up)
  wg/wu [14, 128, 4096] bf16 pre-tiled (lhsT [k128, i128] stationary;
                                        wg[i, p, k*128+m] = Wg.T[k*128+p, i*128+m])
  wd  [32, 128, 1792] bf16 pre-tiled   (lhsT [i128, h128] stationary)
  out [H, T] bf16 partial              (dT; host reduces in f32 + transposes)

Weight DMAs are contiguous per partition (pre-tiled on host) so each is a
single-block-per-partition SWDGE descriptor set. DMA issue is spread over
engines: xT on Sync, weights on Scalar, wd on GpSimd (first three on Vector,
mid-phase, to keep startup bandwidth for the critical xt/wg stream), outputs
on GpSimd (Sync for the last q, where that queue is idle and HWDGE latency
shortens the final drain).

Startup: the PE warms its HAM clock-gate with a short run of dummy matmuls
on a zeroed tile while the first real weight/activation chunks stream in,
so the real matmul stream starts at full clock.
"""

import sys

if "/opt/trn_rl_repo" not in sys.path:
    sys.path.insert(0, "/opt/trn_rl_repo")

import numpy as np
import ml_dtypes

H = 4096          # hidden
I_FULL = 14336    # intermediate
T = 4096          # tokens (2*2048)
NCORES = 8
ISH = I_FULL // NCORES   # 1792 per-core intermediate slice
P = 128
QT = 1024         # tokens per outer block
NQ = T // QT      # 4
KT = H // P       # 32 contraction tiles for gate/up
IT = ISH // P     # 14 contraction tiles for down
HT = H // P       # 32 output-row tiles for down
NF = 512          # matmul moving free-dim (one PSUM bank of f32)
NWARM = 52        # dummy matmuls to span preamble->first-data and warm HAM

_BUILT = {}


def _build():
    if "nc" in _BUILT:
        return _BUILT["nc"]
    from concourse import bacc
    import concourse.mybir as mybir
    import concourse.tile as tile
    from contextlib import ExitStack

    bf = mybir.dt.bfloat16
    f32 = mybir.dt.float32
    nc = bacc.Bacc(
        "TRN2",
        target_bir_lowering=False,
        debug=False,
        enable_asserts=False,
        num_devices=NCORES,
    )

    xT = nc.dram_tensor("xT", [H, T], bf, kind="ExternalInput").ap()
    wg = nc.dram_tensor("wg", [IT, P, KT * P], bf, kind="ExternalInput").ap()
    wu = nc.dram_tensor("wu", [IT, P, KT * P], bf, kind="ExternalInput").ap()
    wd = nc.dram_tensor("wd", [HT, P, IT * P], bf, kind="ExternalInput").ap()
    out = nc.dram_tensor("out", [H, T], bf, kind="ExternalOutput").ap()

    # [p, k, t] view: per-partition rows stay contiguous in t
    x_r = xT.rearrange("(k p) t -> p k t", p=P)     # [128, 32, 4096]

    with tile.TileContext(nc) as tc, ExitStack() as ctx:
        xt_pool = ctx.enter_context(tc.tile_pool(name="xt", bufs=KT + 2))
        wg_pool = ctx.enter_context(tc.tile_pool(name="wg", bufs=3))
        wu_pool = ctx.enter_context(tc.tile_pool(name="wu", bufs=2))
        wd_pool = ctx.enter_context(tc.tile_pool(name="wd", bufs=3))
        at_pool = ctx.enter_context(tc.tile_pool(name="at", bufs=IT))
        tmp_pool = ctx.enter_context(tc.tile_pool(name="tmp", bufs=2))
        dst_pool = ctx.enter_context(tc.tile_pool(name="dst", bufs=2))
        warm_pool = ctx.enter_context(tc.tile_pool(name="warm", bufs=1))
        pg_pool = ctx.enter_context(tc.tile_pool(name="pg", bufs=1, space="PSUM"))
        pu_pool = ctx.enter_context(tc.tile_pool(name="pu", bufs=1, space="PSUM"))
        pd_pool = ctx.enter_context(tc.tile_pool(name="pd", bufs=2, space="PSUM"))

        # ---- PE warmup: dummy matmuls on a zeroed tile while the first
        # real chunks stream from HBM. Keeps the PE busy from right after
        # the preamble barrier so the HAM clock-gate opens (K=8/8) before
        # the real stream begins, and hides the first-DMA latency. The
        # dummies target the first real pg tile (its first real matmul
        # restarts the accumulation group, so the garbage is discarded).
        warm_t = warm_pool.tile([P, P], bf)
        nc.vector.memset(warm_t[:], 0.0)
        pg0 = pg_pool.tile([P, QT], f32, name="pg")
        for w in range(NWARM):
            nc.tensor.matmul(
                pg0[:, 0:P], warm_t[:], warm_t[:], start=True, stop=True
            )

        def load_w(pool, src, i):
            t = pool.tile([P, KT, P], bf)
            # src[i] is [128, 4096] contiguous per partition
            nc.scalar.dma_start(out=t[:], in_=src[i].rearrange("p (k m) -> p k m", m=P))
            return t

        early_wd = {}

        for q in range(NQ):
            t0 = q * QT

            # first gate/up weights go out before the xT block so the PE can
            # start as soon as xt[0] lands; at kernel start, stage the first
            # TWO i-tiles' weights in interleaved k-chunks so every k-group
            # of the startup double-interleave has its weights early
            if q == 0:
                wg_t = wg_pool.tile([P, KT, P], bf)
                wu_t = wu_pool.tile([P, KT, P], bf)
                wg_t1 = wg_pool.tile([P, KT, P], bf, name="wg_t")
                wu_t1 = wu_pool.tile([P, KT, P], bf, name="wu_t")
                views = [
                    (wg_t, wg[0].rearrange("p (k m) -> p k m", m=P)),
                    (wu_t, wu[0].rearrange("p (k m) -> p k m", m=P)),
                    (wg_t1, wg[1].rearrange("p (k m) -> p k m", m=P)),
                    (wu_t1, wu[1].rearrange("p (k m) -> p k m", m=P)),
                ]
                for lo, hi in ((0, 2), (2, 6), (6, 14), (14, 23), (23, KT)):
                    for t, v in views:
                        nc.scalar.dma_start(out=t[:, lo:hi, :], in_=v[:, lo:hi, :])
            else:
                wg_t = load_w(wg_pool, wg, 0)
                wu_t = load_w(wu_pool, wu, 0)

            # stage this block's activations: 32 k-tiles of [128, QT]
            xts = []
            for k in range(KT):
                xt_t = xt_pool.tile([P, QT], bf)
                nc.sync.dma_start(out=xt_t[:], in_=x_r[:, k, t0 : t0 + QT])
                xts.append(xt_t)

            if q == 0:
                # first wd prefetches ride the sync ring BEHIND this q's xt
                # tiles: same-queue FIFO keeps them off the HBM-bound startup
                # window without a separate pacing mechanism
                for hh in range(3):
                    wd_t = wd_pool.tile([P, IT, P], bf, name="wd_t")
                    nc.sync.dma_start(
                        out=wd_t[:], in_=wd[hh].rearrange("p (i m) -> p i m", m=P)
                    )
                    early_wd[hh] = wd_t

            # ---- gate/up + silu*mul, producing aT[i] tiles ----
            ats = []
            if q == 0:
                # kernel start: the whole first pass is HBM-arrival-bound
                # (xt 8MB + weights must stream in). Interleave the first
                # TWO i-tiles per k so PE consumption (~1.7us/k-tile) stays
                # well behind arrival (~1.2us/k-tile) — no starvation, and
                # the HAM clock-gate stays open. i1's accumulators borrow
                # the down-projection PSUM slots, which are idle until the
                # down phase and match the tile shape exactly.
                pg = pg0
                pu = pu_pool.tile([P, QT], f32)
                pg1 = pd_pool.tile([P, QT], f32, name="pd")
                pu1 = pd_pool.tile([P, QT], f32, name="pd")
                tmp = tmp_pool.tile([P, QT], bf)
                tmp1 = tmp_pool.tile([P, QT], bf, name="tmp")
                for k in range(KT):
                    for w_t, ps in (
                        (wg_t, pg), (wu_t, pu), (wg_t1, pg1), (wu_t1, pu1)
                    ):
                        if k == KT - 1 and ps is pu:
                            # silu issued before the final u matmuls so the
                            # pg bank frees as early as possible for i=2
                            nc.scalar.activation(
                                tmp[:], pg[:], mybir.ActivationFunctionType.Silu
                            )
                        if k == KT - 1 and ps is pu1:
                            nc.scalar.activation(
                                tmp1[:], pg1[:], mybir.ActivationFunctionType.Silu
                            )
                        for n in range(QT // NF):
                            nc.tensor.matmul(
                                ps[:, n * NF : (n + 1) * NF],
                                w_t[:, k, :],
                                xts[k][:, n * NF : (n + 1) * NF],
                                start=(k == 0),
                                stop=(k == KT - 1),
                            )
                at = at_pool.tile([P, QT], bf)
                nc.vector.tensor_tensor(at[:], tmp[:], pu[:], mybir.AluOpType.mult)
                ats.append(at)
                at = at_pool.tile([P, QT], bf, name="at")
                nc.vector.tensor_tensor(at[:], tmp1[:], pu1[:], mybir.AluOpType.mult)
                ats.append(at)
            for i in range(0 if q else 2, IT):
                if q > 0 and i == 0:
                    pass
                else:
                    wg_t = load_w(wg_pool, wg, i)
                    wu_t = load_w(wu_pool, wu, i)
                pg = pg_pool.tile([P, QT], f32)
                if True:
                    for k in range(KT):
                        for n in range(QT // NF):
                            nc.tensor.matmul(
                                pg[:, n * NF : (n + 1) * NF],
                                wg_t[:, k, :],
                                xts[k][:, n * NF : (n + 1) * NF],
                                start=(k == 0),
                                stop=(k == KT - 1),
                            )
                    # silu(g) on ScalarE while the u matmuls run
                    tmp = tmp_pool.tile([P, QT], bf)
                    nc.scalar.activation(
                        tmp[:], pg[:], mybir.ActivationFunctionType.Silu
                    )
                    pu = pu_pool.tile([P, QT], f32)
                    for k in range(KT):
                        for n in range(QT // NF):
                            nc.tensor.matmul(
                                pu[:, n * NF : (n + 1) * NF],
                                wu_t[:, k, :],
                                xts[k][:, n * NF : (n + 1) * NF],
                                start=(k == 0),
                                stop=(k == KT - 1),
                            )
                at = at_pool.tile([P, QT], bf)
                nc.vector.tensor_tensor(
                    at[:], tmp[:], pu[:], mybir.AluOpType.mult
                )
                ats.append(at)

            # ---- down projection: dT[h, t] partial ----
            last_q = q == NQ - 1
            for h in range(HT):
                h0 = h * P
                if q == 0 and h in early_wd:
                    wd_t = early_wd.pop(h)
                else:
                    wd_t = wd_pool.tile([P, IT, P], bf)
                    nc.gpsimd.dma_start(
                        out=wd_t[:], in_=wd[h].rearrange("p (i m) -> p i m", m=P)
                    )
                pd = pd_pool.tile([P, QT], f32)
                if last_q and h == HT - 1:
                    # tail: finish the early chunks first so their copies +
                    # stores overlap the later matmuls; the final chunk is
                    # small (short HBM write-receipt) and sits in the other
                    # pd buffer so its matmuls don't share a PSUM bank with
                    # the chunk-2 copy
                    pd2 = pd_pool.tile([P, QT], f32, name="pd")
                    for c0, c1, ps in ((0, 512, pd), (512, 768, pd), (768, 1024, pd2)):
                        for i in range(IT):
                            nc.tensor.matmul(
                                ps[:, c0:c1],
                                wd_t[:, i, :],
                                ats[i][:, c0:c1],
                                start=(i == 0),
                                stop=(i == IT - 1),
                            )
                        dst = dst_pool.tile([P, NF], bf)
                        nc.vector.tensor_copy(dst[:, 0 : c1 - c0], ps[:, c0:c1])
                        if c1 == QT:
                            # very last store: split across the two HWDGE
                            # rings so the HBM write receipts overlap
                            cm = (c0 + c1) // 2
                            nc.sync.dma_start(
                                out=out[h0 : h0 + P, t0 + c0 : t0 + cm],
                                in_=dst[:, 0 : cm - c0],
                            )
                            nc.scalar.dma_start(
                                out=out[h0 : h0 + P, t0 + cm : t0 + c1],
                                in_=dst[:, cm - c0 : c1 - c0],
                            )
                        else:
                            nc.sync.dma_start(
                                out=out[h0 : h0 + P, t0 + c0 : t0 + c1],
                                in_=dst[:, 0 : c1 - c0],
                            )
                else:
                    for i in range(IT):
                        for n in range(QT // NF):
                            nc.tensor.matmul(
                                pd[:, n * NF : (n + 1) * NF],
                                wd_t[:, i, :],
                                ats[i][:, n * NF : (n + 1) * NF],
                                start=(i == 0),
                                stop=(i == IT - 1),
                            )
                    dst = dst_pool.tile([P, QT], bf)
                    nc.vector.tensor_copy(dst[:], pd[:])
                    # output DMAs on gpsimd so their waits don't stall input
                    # loads; on the last q the sync queue is idle and its
                    # HWDGE path has lower latency
                    eng = nc.sync if last_q else nc.gpsimd
                    eng.dma_start(
                        out=out[h0 : h0 + P, t0 : t0 + QT], in_=dst[:]
                    )

    nc.compile()
    _BUILT["nc"] = nc
    return nc


def _prep_inputs(x, Wg, Wu, Wd):
    bf = ml_dtypes.bfloat16
    xTn = x.reshape(T, H).T.astype(bf, order="C")        # [H, T]
    # single-pass cast + shard + pre-tile:
    #   wg[c][i, p, k*128+m] = Wg.T[k*128+p, c*1792 + i*128+m]
    wg_all = np.ascontiguousarray(
        Wg.reshape(NCORES, IT, P, KT, P).transpose(0, 1, 4, 3, 2), dtype=bf
    ).reshape(NCORES, IT, P, KT * P)
    wu_all = np.ascontiguousarray(
        Wu.reshape(NCORES, IT, P, KT, P).transpose(0, 1, 4, 3, 2), dtype=bf
    ).reshape(NCORES, IT, P, KT * P)
    #   wd[c][h, p, i*128+m] = Wd.T[c*1792 + i*128+p, h*128+m]
    wd_all = np.ascontiguousarray(
        Wd.reshape(HT, P, NCORES, IT, P).transpose(2, 0, 4, 3, 1), dtype=bf
    ).reshape(NCORES, HT, P, IT * P)
    return [
        {"xT": xTn, "wg": wg_all[c], "wu": wu_all[c], "wd": wd_all[c]}
        for c in range(NCORES)
    ]


def _run(in_maps, **kw):
    from concourse.bass_utils import run_bass_kernel_spmd

    nc = _build()
    return run_bass_kernel_spmd(nc, in_maps, core_ids=list(range(NCORES)), **kw)


def _gather(results, batch_shape):
    acc = results[0]["out"].astype(np.float32)
    for r in results[1:]:
        acc += r["out"].astype(np.float32)
    return np.ascontiguousarray(acc.T).reshape(batch_shape)


def kernel(x, Wg, Wu, Wd):
    x = np.asarray(x)
    in_maps = _prep_inputs(
        np.asarray(x, dtype=np.float32),
        np.asarray(Wg, dtype=np.float32),
        np.asarray(Wu, dtype=np.float32),
        np.asarray(Wd, dtype=np.float32),
    )
    res = _run(in_maps)
    return _gather(res.results, x.shape)
